# revision 42
# baseline (speedup 1.0000x reference)
# GQA attention (B=2, T=2048, DM=2048, H=16, KV=4, D=128) + RoPE + causal mask
# on 8 TRN2 NeuronCores.
#
# Sharding: rank r = (batch b = r//4, kv-group g = r%4): q-heads 4g..4g+3 and
# kv-head g for batch b; SDPA head-sharded; per-HEAD AllGathers within each
# 4-rank batch group; o_proj column-sharded (each rank multiplies gathered
# O^T by its 512-column Wo slice); host transposes and concatenates.
#
# The phase-2 critical path is the serial collective chain, so the chain is
# started as early as possible and kept trigger-paced: attention head h is
# woven at EMISSION level (the Tile scheduler keeps per-engine emission
# order) into q-projection pass h+1, so AG(h0) fires ~100us in instead of
# ~140.  PSUM plan: q passes use only 2 banks (psQ, half-passes), leaving
# psS(2)+psAV(4) for the woven attention; the k+v pass borrows all 8.
# Consume rounds (ACT-free o_proj matmuls) weave into attention h3 to fill
# the exp-bound kt loops; their DVE ops are chained with explicit deps so a
# late collective can never head-of-line block the DVE queue.

import os
import sys

import numpy as np

for _p in ("/opt/trn_rl_repo", "/root/.axon_site/_ro/trn_rl_repo"):
    if os.path.isdir(_p) and _p not in sys.path:
        sys.path.insert(0, _p)

import ml_dtypes

import concourse.bass as bass
import concourse.mybir as mybir
import concourse.tile as tile
import concourse.masks as masks
from concourse import bacc
from concourse.bass_utils import run_bass_kernel_spmd
from concourse.tile import add_dep_helper

BF16 = ml_dtypes.bfloat16

B, T, DM = 2, 2048, 2048
H, KV, D = 16, 4, 128
NH = H // KV  # 4 local q heads per rank
P = 128
NCORES = 8
NT = T // 512  # 4 free-dim chunks of 512
NDC = DM // P  # 16 contraction chunks
SCALE = float(D) ** -0.5
ROPE_BASE = 10000.0

_bf = mybir.dt.bfloat16
_f32 = mybir.dt.float32
_EXP = mybir.ActivationFunctionType.Exp


def _host_tables():
    inv = 1.0 / (ROPE_BASE ** (np.arange(0, D, 2, dtype=np.float32) / D))
    t = np.arange(T, dtype=np.float32)
    fr = np.outer(t, inv)  # [T, 64]
    emb = np.concatenate([fr, fr], axis=-1)  # [T, D]
    cosT = np.ascontiguousarray(np.cos(emb).T).astype(BF16)  # [D, T]
    sinT = np.sin(emb).T
    sinTs = np.concatenate([-sinT[:64], sinT[64:]], axis=0)
    sinTs = np.ascontiguousarray(sinTs).astype(BF16)
    i = np.arange(P)[:, None]
    j = np.arange(P)[None, :]
    tri = (i <= j).astype(BF16)  # [128, 128] upper-triangular keep-mask
    return cosT, sinTs, tri


def _weave(ga, na, gb, nb):
    # Emit two generators' units interleaved proportionally, then run tails.
    ia = ib = 0
    while ia < na or ib < nb:
        if ib >= nb or (ia < na and ia * nb <= ib * na):
            next(ga, None)
            ia += 1
        else:
            next(gb, None)
            ib += 1
    for _ in ga:
        pass
    for _ in gb:
        pass


def _drain(g):
    for _ in g:
        pass


def _chain(*gens):
    for g in gens:
        yield from g


def _kernel_body(tc, nc, xT, wq, wk, wv, wo, cosT, sinTs, tri, out):
    with (
        tc.tile_pool(name="cpool", bufs=1) as cpool,
        tc.tile_pool(name="qkvp", bufs=1) as qkvp,
        tc.tile_pool(name="wop", bufs=1) as wop,
        tc.tile_pool(name="att", bufs=3) as ap_,
        tc.tile_pool(name="expp", bufs=8) as expp,
        tc.tile_pool(name="accp", bufs=2) as accp,
        tc.tile_pool(name="psS", bufs=2, space="PSUM") as psS,
        tc.tile_pool(name="psAV", bufs=1, space="PSUM") as psAV,
        tc.tile_pool(name="dram", bufs=1, space="DRAM") as dram,
    ):
        # ---- persistent SBUF tensors ----
        tri_sb = cpool.tile([P, P], _bf, name="tri")
        ones_sb = cpool.tile([P, P], _bf, name="ones")

        qT = qkvp.tile([P, NH, T], _bf, name="qT")
        kT = qkvp.tile([P, T], _bf, name="kT")
        v_sb = [qkvp.tile([P, D], _bf, name=f"v{tt}") for tt in range(NDC)]

        wo_sb = wop.tile([P, H, 512], _bf, name="wo_sb")
        oacc = [
            [wop.tile([P, 512], _bf, name=f"oacc{mt}_{tcn}") for tcn in range(NT)]
            for mt in range(4)
        ]

        # per-HEAD AllGather buffers: in = my 4 oq chunks, out = all 4 ranks'
        ag_in = dram.tile([NH, NT, P, 512], _bf, name="ag_in")
        ag_out = [
            dram.tile([KV, NT, P, 512], _bf, name=f"ag_out{h}") for h in range(NH)
        ]
        warm_in = dram.tile([P, 8], _bf, name="warm_in")
        warm_out = dram.tile([KV, P, 8], _bf, name="warm_out")

        dve_order = {"prev": None}

        def ag_trig(h):
            nc.gpsimd.collective_compute(
                "AllGather",
                mybir.AluOpType.bypass,
                replica_groups=[[0, 1, 2, 3], [4, 5, 6, 7]],
                ins=[ag_in[h].opt()],
                outs=[ag_out[h].opt()],
            )

        def issue_load(h, agsb):
            # one coalesced DMA per q-chunk of the AllGather output
            sbs = []
            for l in range(NT):
                t_ = agsb.tile(
                    [P, KV, 512], _bf, tag="agbig", name=f"ag{h}{l}", bufs=16
                )
                nc.sync.dma_start(
                    t_[:], ag_out[h][:, l].rearrange("i p f -> p i f")
                )
                sbs.append(t_)
            return sbs

        def consume_gen(ph, sbs, psO):
            # o_proj contribution of global heads {4i+ph} into oacc, one
            # (qc, mt) psum round per unit; DVE ops chained by explicit deps
            # so the scheduler cannot place them ahead of attention DVE work.
            for l in range(NT):
                for mt in range(4):
                    pos = psO.tile(
                        [P, 512], _f32,
                        tag=f"pos{(ph * 16 + l * 4 + mt) % 2}",
                        name=f"pos{ph}_{l}_{mt}",
                    )
                    for i in range(KV):
                        nc.tensor.matmul(
                            pos[:],
                            wo_sb[:, 4 * i + ph, mt * P : (mt + 1) * P],
                            sbs[l][:, i, :],
                            start=(i == 0),
                            stop=(i == KV - 1),
                        )
                    if ph == 0:
                        d = nc.vector.tensor_copy(oacc[mt][l][:], pos[:])
                    else:
                        d = nc.vector.tensor_add(
                            oacc[mt][l][:], oacc[mt][l][:], pos[:]
                        )
                    if dve_order["prev"] is not None:
                        add_dep_helper(
                            d.ins, dve_order["prev"].ins,
                            reason="consume DVE after newest DVE",
                        )
                    dve_order["prev"] = d
                    if ph == NH - 1:
                        nc.scalar.dma_start(
                            out[mt * P : (mt + 1) * P, l * 512 : (l + 1) * 512],
                            oacc[mt][l][:],
                        )
                    yield

        def att_head_gen(h):
            # full head in ONE causal group (4 AV accumulators on av0-av3),
            # one kt iteration per unit; fires this head's AllGather at the
            # end so the collective chain is trigger-paced.
            g = [0, 1, 2, 3]
            acc = accp.tile([P, 4 * 512], _bf, tag="acc", name=f"acc{h}")
            avs = {
                qc: psAV.tile([P, 512], _f32, tag=f"av{i}", name=f"av{h}_{qc}")
                for i, qc in enumerate(g)
            }

            def finish(qc, h=h, acc=acc, avs=avs):
                # denominator (k-partition sum + broadcast via ones matmul),
                # fast DVE reciprocal, normalize, ship to the AG input.
                dps = psS.tile([P, 512], _f32, tag="s", name="sden")
                nc.tensor.matmul(
                    dps[:],
                    ones_sb[:],
                    acc[:, qc * 512 : (qc + 1) * 512],
                    start=True,
                    stop=True,
                )
                rec = ap_.tile([P, 512], _f32, tag="rec", name="rec")
                nc.vector.reciprocal_approx_fast(rec[:], dps[:])
                oq = ap_.tile([P, 512], _bf, tag="oq", name="oq")
                m = nc.vector.tensor_mul(oq[:], avs[qc][:], rec[:])
                dve_order["prev"] = m
                # scalar ring: the sync ring carries the (long-waiting)
                # AG-output loads, which must not delay this write->trigger
                nc.scalar.dma_start(ag_in[h, qc], oq[:])

            pend_av = None
            for kt in range(4 * max(g) + 4):
                lhs_k = kT[:, kt * P : (kt + 1) * P]
                valid = [qc for qc in g if kt <= 4 * qc + 3]
                exs = {}
                for qc in valid:
                    bound = kt // 4 == qc
                    off = 128 * (kt % 4) if bound else 0
                    w = 512 - off
                    qs = slice(qc * 512 + off, (qc + 1) * 512)
                    ps = psS.tile([P, 512], _f32, tag="s", name="s")
                    nc.tensor.matmul(
                        ps[:, :w], lhs_k, qT[:, h, qs], start=True, stop=True
                    )
                    ex = expp.tile([P, 512], _bf, tag="exp", name="ex")
                    nc.scalar.activation(ex[:, :w], ps[:, :w], _EXP, scale=SCALE)
                    if bound:
                        nc.vector.tensor_mul(ex[:, :P], ex[:, :P], tri_sb[:])
                    a0 = qc * 512
                    if kt == 0:
                        aa = nc.vector.tensor_copy(
                            acc[:, a0 : a0 + 512], ex[:, :512]
                        )
                    else:
                        aa = nc.vector.tensor_add(
                            acc[:, a0 + off : a0 + 512],
                            acc[:, a0 + off : a0 + 512],
                            ex[:, :w],
                        )
                    dve_order["prev"] = aa
                    exs[qc] = (ex, off, w)
                # AV runs one kt behind scores so the PE never waits on
                # ScalarE's exp round-trip
                if pend_av is not None:
                    pkt, pexs = pend_av
                    for qc, (exw, off, w) in pexs.items():
                        nc.tensor.matmul(
                            avs[qc][:, off:],
                            v_sb[pkt][:],
                            exw[:, :w],
                            start=(pkt == 0),
                            stop=(pkt == 4 * qc + 3),
                        )
                    for qc in pexs:
                        if pkt == 4 * qc + 3:
                            finish(qc)
                pend_av = (kt, exs)
                yield
            pkt, pexs = pend_av
            for qc, (exw, off, w) in pexs.items():
                nc.tensor.matmul(
                    avs[qc][:, off:],
                    v_sb[pkt][:],
                    exw[:, :w],
                    start=(pkt == 0),
                    stop=(pkt == 4 * qc + 3),
                )
            for qc in pexs:
                if pkt == 4 * qc + 3:
                    finish(qc)
            ag_trig(h)

        # ---- QKV projections (+ fused RoPE), attention heads woven in ----
        with (
            tc.tile_pool(name="xpool", bufs=1) as xpool,
            tc.tile_pool(name="wpool", bufs=1) as wpool,
            tc.tile_pool(name="psQ", bufs=1, space="PSUM") as psQ,
            tc.tile_pool(name="rope", bufs=2) as rp,
        ):
            x_sb = [
                xpool.tile([P, T], _bf, tag=f"x{dc}", name=f"x{dc}")
                for dc in range(NDC)
            ]
            wq_sb = wpool.tile([P, NDC, NH * D], _bf, name="wq_sb")
            wk_sb = wpool.tile([P, NDC, D], _bf, name="wk_sb")
            wv_sb = wpool.tile([P, NDC, D], _bf, name="wv_sb")
            cos_sb = wpool.tile([P, T], _bf, name="cos_sb")
            sin_sb = wpool.tile([P, T], _bf, name="sin_sb")
            ident = wpool.tile([P, P], _bf, name="ident")
            scr = wpool.tile([P, 8], _f32, name="scr")
            vT_all = wpool.tile([P, NT, 512], _bf, name="vT_all")

            # x streams on the sync HWDGE ring; weights on the scalar ring;
            # wk/wv split so the first matmuls wait only on first chunks.
            nc.scalar.dma_start(wk_sb[:, 0:4], wk[:, 0:4])
            nc.scalar.dma_start(wv_sb[:, 0:4], wv[:, 0:4])
            nc.sync.dma_start(x_sb[0][:], xT[0:P, :])
            nc.scalar.dma_start(x_sb[1][:], xT[P : 2 * P, :])
            nc.scalar.dma_start(wk_sb[:, 4:], wk[:, 4:])
            nc.scalar.dma_start(wv_sb[:, 4:], wv[:, 4:])
            for dc in range(2, NDC):
                eng = nc.sync if dc % 2 == 0 else nc.scalar
                eng.dma_start(x_sb[dc][:], xT[dc * P : (dc + 1) * P, :])
            nc.scalar.dma_start(wq_sb[:, 0:8], wq[:, 0:8])
            nc.scalar.dma_start(cos_sb[:], cosT)
            nc.scalar.dma_start(sin_sb[:], sinTs)
            nc.scalar.dma_start(wq_sb[:, 8:], wq[:, 8:])
            nc.scalar.dma_start(tri_sb[:], tri)
            nc.scalar.dma_start(wo_sb[:], wo)
            nc.vector.memset(ones_sb[:], 1.0)
            masks.make_identity(nc, ident[:])

            # preload the exp table + warm the collectives path under x load
            nc.scalar.activation(scr[:], ones_sb[:, :8], _EXP)
            nc.scalar.dma_start(warm_in[:], ones_sb[:, :8])
            nc.gpsimd.collective_compute(
                "AllGather",
                mybir.AluOpType.bypass,
                replica_groups=[[0, 1, 2, 3], [4, 5, 6, 7]],
                ins=[warm_in.opt()],
                outs=[warm_out.opt()],
            )

            def rope(ps, tcn, dst):
                ts = slice(tcn * 512, (tcn + 1) * 512)
                src = rp.tile([P, 512], _bf, tag="rsrc", name="rsrc")
                nc.scalar.copy(src[:], ps[:])
                swp = rp.tile([P, 512], _bf, tag="rswp", name="rswp")
                # sync ring: idle once x lands, and all phase-1 sync
                # emission precedes the AG-output loads, so the swaps no
                # longer contend with oq writes on the scalar ring (which
                # stalled the rope DVE 50+us at head boundaries)
                nc.sync.dma_start(swp[0:64, :], src[64:128, :])
                nc.sync.dma_start(swp[64:128, :], src[0:64, :])
                nc.vector.tensor_mul(src[:], src[:], cos_sb[:, ts])
                nc.vector.tensor_mul(swp[:], swp[:], sin_sb[:, ts])
                nc.vector.tensor_add(dst, src[:], swp[:])

            # k+v shared pass (8 accumulators: k on psAV av0-3, v on
            # psS s,s + psQ TA0,TA1) consumes x chunks at streaming rate
            kacc = [
                psAV.tile([P, 512], _f32, tag=f"av{t}", name=f"kacc{t}")
                for t in range(NT)
            ]
            vacc = [
                psS.tile([P, 512], _f32, tag="s", name="vacc0"),
                psS.tile([P, 512], _f32, tag="s", name="vacc1"),
                psQ.tile([P, 512], _f32, tag="TA0", name="vacc2"),
                psQ.tile([P, 512], _f32, tag="TA1", name="vacc3"),
            ]
            for dc in range(NDC):
                for lhs, pss in (
                    (wk_sb[:, dc, :], kacc),
                    (wv_sb[:, dc, :], vacc),
                ):
                    for tcn in range(NT):
                        nc.tensor.matmul(
                            pss[tcn][:],
                            lhs,
                            x_sb[dc][:, tcn * 512 : (tcn + 1) * 512],
                            start=(dc == 0),
                            stop=(dc == NDC - 1),
                        )
            for tcn in range(NT):
                rope(kacc[tcn], tcn, kT[:, tcn * 512 : (tcn + 1) * 512])
                nc.scalar.copy(vT_all[:, tcn], vacc[tcn][:])
            # v transposes into [t, d] tiles via the freed psQ banks
            for tcn in range(NT):
                for sub in range(4):
                    tt = tcn * 4 + sub
                    ptr = psQ.tile([P, P], _bf, tag=f"TA{sub % 2}", name="vtr")
                    nc.tensor.transpose(
                        ptr[:], vT_all[:, tcn, sub * P : (sub + 1) * P], ident[:]
                    )
                    nc.vector.tensor_copy(v_sb[tt][:], ptr[:])

            # q passes: TWO psQ accumulators (half-passes), leaving
            # psS/psAV free for the woven attention heads
            def wproj1_gen(j):
                for tcns in ((0, 1), (2, 3)):
                    pss = {
                        tcn: psQ.tile(
                            [P, 512], _f32, tag=f"TA{tcn % 2}", name=f"q{j}{tcn}"
                        )
                        for tcn in tcns
                    }
                    for dc in range(NDC):
                        lhs = wq_sb[:, dc, j * P : (j + 1) * P]
                        for tcn in tcns:
                            nc.tensor.matmul(
                                pss[tcn][:],
                                lhs,
                                x_sb[dc][:, tcn * 512 : (tcn + 1) * 512],
                                start=(dc == 0),
                                stop=(dc == NDC - 1),
                            )
                        yield
                    for tcn in tcns:
                        rope(pss[tcn], tcn, qT[:, j, tcn * 512 : (tcn + 1) * 512])

            _drain(wproj1_gen(0))
            # attention head h weaves into q pass h+1: AG(h) fires as soon
            # as head h's kt loop drains, starting the collective chain
            # ~40us earlier than a serial schedule
            _weave(att_head_gen(0), 16, wproj1_gen(1), 32)
            _weave(att_head_gen(1), 16, wproj1_gen(2), 32)
            _weave(att_head_gen(2), 16, wproj1_gen(3), 32)

        # ---- attention tail + pipelined o_proj consumes ----
        with (
            tc.tile_pool(name="psO", bufs=1, space="PSUM") as psO,
            tc.tile_pool(name="agsb", bufs=1) as agsb,
        ):
            sb0 = issue_load(0, agsb)
            sb1 = issue_load(1, agsb)
            sb2 = issue_load(2, agsb)
            # only consume u0 weaves into attention h3, so h3's kt loop
            # (and with it the LAST AllGather trigger) finishes ~20us
            # earlier; c1/c2 then fill the PE while AG(3) lands
            _weave(att_head_gen(3), 16, consume_gen(0, sb0, psO), 16)
            sb3 = issue_load(3, agsb)
            _drain(consume_gen(1, sb1, psO))
            _drain(consume_gen(2, sb2, psO))
            _drain(consume_gen(3, sb3, psO))


def build_nc():
    nc = bacc.Bacc(
        "TRN2", target_bir_lowering=False, debug=False, num_devices=NCORES
    )
    xT = nc.dram_tensor("xT", [DM, T], _bf, kind="ExternalInput").ap()
    wq = nc.dram_tensor("wq", [P, NDC, NH * D], _bf, kind="ExternalInput").ap()
    wk = nc.dram_tensor("wk", [P, NDC, D], _bf, kind="ExternalInput").ap()
    wv = nc.dram_tensor("wv", [P, NDC, D], _bf, kind="ExternalInput").ap()
    wo = nc.dram_tensor("wo", [P, H, 512], _bf, kind="ExternalInput").ap()
    cosT = nc.dram_tensor("cosT", [D, T], _bf, kind="ExternalInput").ap()
    sinTs = nc.dram_tensor("sinTs", [D, T], _bf, kind="ExternalInput").ap()
    tri = nc.dram_tensor("tri", [P, P], _bf, kind="ExternalInput").ap()
    out = nc.dram_tensor("out", [512, T], _bf, kind="ExternalOutput").ap()
    with tile.TileContext(nc) as tc:
        _kernel_body(tc, nc, xT, wq, wk, wv, wo, cosT, sinTs, tri, out)
    nc.finalize()
    return nc


def _chunked(w, cols):
    return np.ascontiguousarray(
        w.reshape(NDC, P, cols).transpose(1, 0, 2)
    ).astype(BF16)


def make_in_maps(x, Wq, Wk, Wv, Wo):
    cosT, sinTs, tri = _host_tables()
    xTb = [np.ascontiguousarray(x[b].T).astype(BF16) for b in range(B)]
    wg = []
    for g in range(KV):
        wg.append(
            {
                "wq": _chunked(Wq[:, g * NH * D : (g + 1) * NH * D], NH * D),
                "wk": _chunked(Wk[:, g * D : (g + 1) * D], D),
                "wv": _chunked(Wv[:, g * D : (g + 1) * D], D),
                "wo": np.ascontiguousarray(
                    Wo[:, g * 512 : (g + 1) * 512].reshape(H, P, 512).transpose(1, 0, 2)
                ).astype(BF16),
            }
        )
    in_maps = []
    for r in range(NCORES):
        b, g = divmod(r, KV)
        m = {"xT": xTb[b], "cosT": cosT, "sinTs": sinTs, "tri": tri}
        m.update(wg[g])
        in_maps.append(m)
    return in_maps


def assemble(results):
    out = np.empty((B, T, DM), np.float32)
    for r in range(NCORES):
        b, g = divmod(r, KV)
        out[b, :, g * 512 : (g + 1) * 512] = results[r]["out"].T.astype(np.float32)
    return out


_NC_CACHE = {}


def get_nc():
    if "nc" not in _NC_CACHE:
        _NC_CACHE["nc"] = build_nc()
    return _NC_CACHE["nc"]


def run(x, Wq, Wk, Wv, Wo, trace=False, taps=False):
    nc = get_nc()
    in_maps = make_in_maps(x, Wq, Wk, Wv, Wo)
    res = run_bass_kernel_spmd(
        nc, in_maps, core_ids=list(range(NCORES)), trace=trace
    )
    return assemble(res.results), res


def kernel(x, Wq, Wk, Wv, Wo, mask=None, **_unused):
    x = np.asarray(x, dtype=np.float32)
    Wq = np.asarray(Wq, dtype=np.float32)
    Wk = np.asarray(Wk, dtype=np.float32)
    Wv = np.asarray(Wv, dtype=np.float32)
    Wo = np.asarray(Wo, dtype=np.float32)
    out, _ = run(x, Wq, Wk, Wv, Wo, trace=False)
    return out


if __name__ == "__main__":
    build_nc()
    print("build OK")
